# revision 1
# baseline (speedup 1.0000x reference)
"""Trainium2 Bass kernel for nn_ContrastiveLoss (patch-level contrastive loss).

Reference math:
  n1 = normalize(normal_embed)  [N,P,D], n2 = normalize(defect_embed) [M,P,D]
  sim_nn[i,j,q] = max_p <n1[i,p,:], n1[j,q,:]>   (max over first arg's patches)
  sim_nd[i,j,q] = max_p <n1[i,p,:], n2[j,q,:]>
  pos_loss = sum_{i<j,q} (1 - sim_nn[i,j,q]) / (npairs*P)
  neg_loss = mean(relu(sim_nd - 0.5))
  loss = pos_loss + neg_loss

Distribution (8 NeuronCores, data-parallel over i):
  Core c owns moving images I_c = {c, 31-c, 15-c, 16+c} (4 normal images,
  normalized, d-major [768, 4*196]).  The j-side (all 32 normalized normal
  images for nn, all 32 defect for nd; d-major [768, 32*196]) is replicated
  and streamed as 128-wide stationary tiles.  For each stationary q-tile
  (128 q's) the kernel runs a 6-chunk PSUM-accumulated matmul against two
  2-image moving batches (free dim 392), then a free-dim reduce_max gives
  max-over-p per q, collected into SBUF slot matrices.  Final on-device
  stage: masked sum (pos) and relu(x-0.5) sum (neg) + a ones-matmul
  partition reduction -> per-core partial sums [1,2].  Host combines:
  loss = 1 - S_pos/(npairs*P) + S_neg/(N*M*P).
"""

import os

import numpy as np

# Problem constants (hardcoded per the contract; kernel.py is self-contained).
N_IMG = 32
P = 196
D = 768
EPS = 1e-8
MARGIN = 0.5
NCORES = 8
KCHUNKS = D // 128          # 6
Q_ALL = N_IMG * P           # 6272
NT = Q_ALL // 128           # 49 stationary q-tiles per side
TGROUP = 4                  # q-tiles per DMA group (512 q's)
NGROUPS = (NT + TGROUP - 1) // TGROUP   # 13 (last group has 1 tile)
QPAD = NGROUPS * TGROUP * 128           # 6656
NPAIRS = N_IMG * (N_IMG - 1) // 2

# Matmul operand dtype: "f32r" (full-rate fp32 on trn2 PE) or "bf16".
MM_DTYPE = os.environ.get("CL_MM_DTYPE", "f32r")
# Bounce DMA'd matmul operands through a DVE copy so matmuls single-wait on
# the DVE semaphore (1 = on). With 0, Bacc's event-semaphore pass legalizes
# multi-waits instead.
BOUNCE = os.environ.get("CL_BOUNCE", "1") == "1"
# Debug bisection knobs: number of sides (2) and DMA groups per side (13).
DBG_SIDES = int(os.environ.get("CL_SIDES", "2"))
DBG_NG = int(os.environ.get("CL_NG", str(NGROUPS)))

_CACHE = {}


def _iset(c):
    """Moving-image set of core c (balanced for the i<j triangle)."""
    return [c, 31 - c, 15 - c, 16 + c]


def _build_nc(mm_dtype_name):
    import concourse.bacc as bacc
    import concourse.mybir as mybir
    import concourse.tile as tile

    f32 = mybir.dt.float32
    mmdt = {"f32r": mybir.dt.float32r, "bf16": mybir.dt.bfloat16}[mm_dtype_name]

    # Bacc (not plain Bass): its compile() runs move_matmul_waits_to_ldweights
    # + generate_event_semaphores, which legalize multi-semaphore waits for
    # the 1-wait-per-instruction ISA constraint on matmul structs.
    nc = bacc.Bacc("TRN2", target_bir_lowering=False, debug=False)

    mov_d = nc.dram_tensor("mov", [D, 4 * P], mmdt, kind="ExternalInput")
    stat_nn_d = nc.dram_tensor("stat_nn", [D, QPAD], mmdt, kind="ExternalInput")
    stat_nd_d = nc.dram_tensor("stat_nd", [D, QPAD], mmdt, kind="ExternalInput")
    wmask_d = nc.dram_tensor("wmask", [128, 4 * NT], f32, kind="ExternalInput")
    out_d = nc.dram_tensor("out", [1, 2], f32, kind="ExternalOutput")

    with tile.TileContext(nc) as tc:
        with (
            tc.tile_pool(name="const", bufs=1) as const_pool,
            tc.tile_pool(name="movp", bufs=1) as mov_pool,
            tc.tile_pool(name="statp", bufs=3) as stat_pool,
            tc.tile_pool(name="slots", bufs=1) as slot_pool,
            tc.tile_pool(name="psum", bufs=3, space="PSUM") as psum_pool,
            tc.tile_pool(name="psum_f", bufs=1, space="PSUM") as psum_f_pool,
        ):
            # Resident moving operand: [128, chunk, 784].
            # The ISA matmul/ldweights structs fit only ONE sync wait, so every
            # matmul dependency must arrive through a single semaphore. All
            # matmul inputs are therefore bounced DRAM -> bounce tile (DMA) ->
            # operand tile (DVE copy): the PE then only ever waits on the DVE
            # semaphore (merged with the PSUM-recycle dep, which is also DVE).
            mov_sb = mov_pool.tile([128, KCHUNKS, 4 * P], mmdt)
            if BOUNCE:
                mov_bounce = mov_pool.tile([128, KCHUNKS, 4 * P], mmdt)
                nc.sync.dma_start(
                    mov_bounce[:],
                    mov_d[:, :].rearrange("(c k) p -> k c p", k=128),
                )
                nc.vector.tensor_copy(mov_sb[:], mov_bounce[:])
            else:
                nc.sync.dma_start(
                    mov_sb[:],
                    mov_d[:, :].rearrange("(c k) p -> k c p", k=128),
                )

            wmask_sb = const_pool.tile([128, 4 * NT], f32)
            nc.sync.dma_start(wmask_sb[:], wmask_d[:, :])

            ones_sb = const_pool.tile([128, 1], f32)
            nc.vector.memset(ones_sb[:], 1.0)

            # Max-over-p slot matrices: column = 4*t + iloc.
            m_slots = [
                slot_pool.tile(
                    [128, 4 * NT], f32, tag=f"mslots{s}", name=f"mslots{s}"
                )
                for s in range(2)
            ]
            junk = [
                slot_pool.tile([128, 4 * NT], f32, tag=f"junk{s}", name=f"junk{s}")
                for s in range(2)
            ]
            acc2 = const_pool.tile([128, 2], f32)
            nc.vector.memset(m_slots[0][:], 0.0)
            nc.vector.memset(m_slots[1][:], 0.0)

            for side, stat_d in enumerate((stat_nn_d, stat_nd_d)[:DBG_SIDES]):
                for g in range(DBG_NG):
                    n_t = min(TGROUP, NT - g * TGROUP)
                    qw = n_t * 128
                    stat_sb = stat_pool.tile(
                        [128, KCHUNKS, TGROUP * 128], mmdt, tag="stat"
                    )
                    src = stat_d[
                        :, g * TGROUP * 128 : g * TGROUP * 128 + qw
                    ].rearrange("(c k) q -> k c q", k=128)
                    if BOUNCE:
                        stat_bounce = stat_pool.tile(
                            [128, KCHUNKS, TGROUP * 128], mmdt, tag="statb"
                        )
                        nc.sync.dma_start(stat_bounce[:, :, 0:qw], src)
                        nc.vector.tensor_copy(
                            stat_sb[:, :, 0:qw], stat_bounce[:, :, 0:qw]
                        )
                    else:
                        nc.sync.dma_start(stat_sb[:, :, 0:qw], src)
                    for tt in range(n_t):
                        t = g * TGROUP + tt
                        ps_a = psum_pool.tile([128, 2 * P], f32, tag="psA")
                        ps_b = psum_pool.tile([128, 2 * P], f32, tag="psB")
                        for c in range(KCHUNKS):
                            lhsT = stat_sb[:, c, tt * 128 : (tt + 1) * 128]
                            nc.tensor.matmul(
                                ps_a[:],
                                lhsT,
                                mov_sb[:, c, 0 : 2 * P],
                                start=(c == 0),
                                stop=(c == KCHUNKS - 1),
                            )
                            nc.tensor.matmul(
                                ps_b[:],
                                lhsT,
                                mov_sb[:, c, 2 * P : 4 * P],
                                start=(c == 0),
                                stop=(c == KCHUNKS - 1),
                            )
                        # max over p (free dim) for each of the 2 images per bank
                        nc.vector.reduce_max(
                            out=m_slots[side][:, 4 * t : 4 * t + 2],
                            in_=ps_a[:].rearrange("k (i p) -> k i p", p=P),
                            axis=mybir.AxisListType.X,
                        )
                        nc.vector.reduce_max(
                            out=m_slots[side][:, 4 * t + 2 : 4 * t + 4],
                            in_=ps_b[:].rearrange("k (i p) -> k i p", p=P),
                            axis=mybir.AxisListType.X,
                        )

            # pos: acc2[:,0] = sum_q mask * m   (standard ops only — the ANT
            # custom-DVE tensor_tensor_reduce fails at runtime via PJRT/axon)
            nc.vector.tensor_mul(junk[0][:], m_slots[0][:], wmask_sb[:])
            nc.vector.reduce_sum(
                out=acc2[:, 0:1], in_=junk[0][:], axis=mybir.AxisListType.X
            )
            # neg: acc2[:,1] = sum_q relu(m - margin)
            nc.vector.tensor_scalar(
                out=junk[1][:],
                in0=m_slots[1][:],
                scalar1=-MARGIN,
                scalar2=0.0,
                op0=mybir.AluOpType.add,
                op1=mybir.AluOpType.max,
            )
            nc.vector.reduce_sum(
                out=acc2[:, 1:2], in_=junk[1][:], axis=mybir.AxisListType.X
            )
            # partition reduction: [1,2] = ones[128,1].T @ acc2[128,2]
            ps_f = psum_f_pool.tile([1, 2], f32)
            nc.tensor.matmul(ps_f[:], ones_sb[:], acc2[:], start=True, stop=True)
            out_sb = const_pool.tile([1, 2], f32)
            nc.vector.tensor_copy(out_sb[:], ps_f[:])
            nc.sync.dma_start(out_d[:, :], out_sb[:])

    nc.compile()
    return nc


def _np_dtype(mm_dtype_name):
    if mm_dtype_name == "bf16":
        import ml_dtypes

        return ml_dtypes.bfloat16
    return np.float32


def _pack_stat(n, dt):
    """[32,P,D] normalized -> d-major [D, QPAD] (j-major q axis), zero-padded."""
    out = np.zeros((D, QPAD), dtype=dt)
    out[:, :Q_ALL] = n.transpose(2, 0, 1).reshape(D, Q_ALL).astype(dt)
    return np.ascontiguousarray(out)


def _build_in_maps(normal_embed, defect_embed, mm_dtype_name):
    dt = _np_dtype(mm_dtype_name)
    x1 = np.asarray(normal_embed, dtype=np.float32)
    x2 = np.asarray(defect_embed, dtype=np.float32)
    n1 = x1 / (np.sqrt(np.sum(x1 * x1, axis=-1, keepdims=True)) + EPS)
    n2 = x2 / (np.sqrt(np.sum(x2 * x2, axis=-1, keepdims=True)) + EPS)

    stat_nn = _pack_stat(n1, dt)
    stat_nd = _pack_stat(n2, dt)

    in_maps = []
    for c in range(NCORES):
        iset = _iset(c)
        mov = np.ascontiguousarray(
            n1[iset].transpose(2, 0, 1).reshape(D, 4 * P).astype(dt)
        )
        # wmask[qw, 4t+iloc] = 1 iff j(q) > i  with q = 128 t + qw
        q = np.arange(NT * 128)
        jq = q // P  # [NT*128]
        wm = np.zeros((128, 4 * NT), dtype=np.float32)
        for iloc, i_img in enumerate(iset):
            col_mask = (jq > i_img).astype(np.float32).reshape(NT, 128).T  # [128,NT]
            wm[:, iloc::4] = col_mask
        in_maps.append(
            {
                "mov": mov,
                "stat_nn": stat_nn,
                "stat_nd": stat_nd,
                "wmask": np.ascontiguousarray(wm),
            }
        )
    return in_maps


def _get_nc():
    key = ("nc", MM_DTYPE, BOUNCE)
    if key not in _CACHE:
        _CACHE[key] = _build_nc(MM_DTYPE)
    return _CACHE[key]


def _run_on_device(in_maps, trace=False):
    from concourse.bass_utils import run_bass_kernel_spmd

    nc = _get_nc()
    return run_bass_kernel_spmd(
        nc, in_maps, core_ids=list(range(NCORES)), trace=trace
    )


def _combine(results):
    s_pos = 0.0
    s_neg = 0.0
    for r in results:
        o = np.asarray(r["out"], dtype=np.float64)
        s_pos += float(o[0, 0])
        s_neg += float(o[0, 1])
    loss = 1.0 - s_pos / (NPAIRS * P) + s_neg / (N_IMG * N_IMG * P)
    return np.float32(loss)


def kernel(normal_embed, defect_embed):
    in_maps = _build_in_maps(normal_embed, defect_embed, MM_DTYPE)
    res = _run_on_device(in_maps, trace=False)
    return _combine(res.results)



# revision 8
# speedup vs baseline: 3.5124x; 3.5124x over previous
"""Trainium2 Bass kernel for nn_ContrastiveLoss (patch-level contrastive loss).

Reference math:
  n1 = normalize(normal_embed)  [N,P,D], n2 = normalize(defect_embed) [M,P,D]
  sim_nn[i,j,q] = max_p <n1[i,p,:], n1[j,q,:]>   (max over first arg's patches)
  sim_nd[i,j,q] = max_p <n1[i,p,:], n2[j,q,:]>
  pos_loss = sum_{i<j,q} (1 - sim_nn[i,j,q]) / (npairs*P)
  neg_loss = mean(relu(sim_nd - 0.5))
  loss = pos_loss + neg_loss

Distribution (8 NeuronCores, data-parallel over i):
  Core c owns moving pairs A=(2c, 2c+1), B=(30-2c, 31-2c). Embeddings are
  normalized on host, scaled by S, quantized to fp8e4m3 and shipped as
  uint8 (bitcast to float8e4 at the matmul). Matmuls run in DoubleRow perf
  mode: each instruction contracts TWO 128-deep k-chunks ([128,2,*] APs),
  so D=768 takes 3 matmuls per PSUM bank at 0.5 cycles/row.

  The j-side is streamed as 128-wide stationary q-tiles against 392-wide
  moving halves (one image pair). The nn side exploits the i<j triangle
  with a core-uniform schedule (single SPMD program):
    - fixed-A: q-tiles 24..48 vs pair A (every core's pair A needs all of
      them since max_c tileof(2c+1) = 22 < 24),
    - fixed-B: q-tiles 47,48 vs pair B,
    - flex: 23 host-packed per-core q-tiles vs BOTH pairs (exactly the
      remaining tiles each core needs: (24-tA)+(47-tB) == 23 for all c);
      the wmask kills the half that doesn't apply,
  plus the full 49-tile sweep for the nd side. Max-over-p reduces are
  interleaved over the DVE and GpSimd engines (4:3) straight out of PSUM,
  two banks per instruction. Final masked sums (pos) and relu sums (neg)
  plus a ones-matmul partition reduction give per-core partials [1,2];
  the host combines them.
"""

import os

import numpy as np

# Problem constants (hardcoded per the contract; kernel.py is self-contained).
N_IMG = 32
P = 196
D = 768
EPS = 1e-8
MARGIN = 0.5
NCORES = 8
NT = N_IMG * P // 128       # 49 stationary q-tiles per side (exact: 6272/128)
NPAIRS = N_IMG * (N_IMG - 1) // 2

# fp8 scale: sims come out multiplied by SCALE^2; undone on host.
SCALE = 16.0

# Triangular-nn schedule (1) vs full NxN (0).
TRI = os.environ.get("CL_TRI", "1") == "1"
# Reduce pipeline per unit, cyclic pattern of modes:
#   V: DVE reduce_max straight from PSUM
#   B: GpSimd tensor_max halves PSUM->SBUF f32, DVE reduce_max finishes
#   C: Act copies PSUM->SBUF bf16, DVE tensor_max halves, DVE reduce finishes
# Default mix keeps DVE/Pool/Act balanced (see notes): ~10 B : 3 C.
RED_PATTERN = os.environ.get("CL_RED", "BBBCBBBCBBBCB")

# Per-core pair bases: pair A = (2c, 2c+1), pair B = (30-2c, 31-2c).
TA = [((2 * c + 1) * P) // 128 for c in range(NCORES)]   # first tile pair A needs
TB = [((31 - 2 * c) * P) // 128 for c in range(NCORES)]  # first tile pair B needs
FIXED_A_T0 = 24   # fixed-A section: tiles 24..48 (>= max(TA)=22)
FIXED_B_T0 = 47   # fixed-B section: tiles 47,48 (>= max(TB)=47)
N_FIXED_A = NT - FIXED_A_T0          # 25
N_FIXED_B = NT - FIXED_B_T0          # 2
N_FLEX = (FIXED_A_T0 - min(TA)) + 0  # 23; invariant checked below
for _c in range(NCORES):
    assert (FIXED_A_T0 - TA[_c]) + (FIXED_B_T0 - TB[_c]) == 23
N_FLEX = 23
NN_SLOTS = N_FIXED_A + N_FLEX        # 48 packed stationary nn tiles

_CACHE = {}


def _pairs(c):
    return (2 * c, 2 * c + 1), (30 - 2 * c, 31 - 2 * c)


def _flex_tiles(c):
    """Per-core flex q-tiles: pair-A leftovers then pair-B leftovers."""
    return list(range(TA[c], FIXED_A_T0)) + list(range(TB[c], FIXED_B_T0))


def _schedule():
    """Core-independent unit list. Each unit: dict with
    side: 0 nn / 1 nd; banks: list of (slot, half) with half 0=pair A cols
    0:392, 1=pair B cols 392:784. Slot indexes 128-col groups of the packed
    stationary tensor for that side."""
    units = []
    if TRI:
        # fixed-A: slots 0..24 = nn tiles 24..48, pair A, two slots per unit
        for u in range(N_FIXED_A // 2):
            units.append({"side": 0, "banks": [(2 * u, 0), (2 * u + 1, 0)]})
        units.append({"side": 0, "banks": [(N_FIXED_A - 1, 0)]})
        # fixed-B: tiles 47,48 = slots 23,24, pair B
        units.append({"side": 0, "banks": [(N_FIXED_A - 2, 1), (N_FIXED_A - 1, 1)]})
        # flex: slots 25..47, both pairs
        for k in range(N_FLEX):
            s = N_FIXED_A + k
            units.append({"side": 0, "banks": [(s, 0), (s, 1)]})
    else:
        for t in range(NT):
            units.append({"side": 0, "banks": [(t, 0), (t, 1)]})
    for t in range(NT):
        units.append({"side": 1, "banks": [(t, 0), (t, 1)]})
    return units


SCHED = _schedule()
NN_UNITS = sum(1 for u in SCHED if u["side"] == 0)
NN_COLS = 4 * NN_UNITS
NCOLS = 4 * len(SCHED)
NN_STAT_COLS = (NN_SLOTS if TRI else NT) * 128
ND_STAT_COLS = NT * 128


def _red_mode_seq(n):
    """Reduce-pipeline mode per unit (cyclic pattern)."""
    return [RED_PATTERN[u % len(RED_PATTERN)] for u in range(n)]


def _build_nc():
    import concourse.bacc as bacc
    import concourse.mybir as mybir
    import concourse.tile as tile

    f32 = mybir.dt.float32
    bf16 = mybir.dt.bfloat16
    u8 = mybir.dt.uint8
    f8 = mybir.dt.float8e4
    DR = mybir.MatmulPerfMode.DoubleRow
    HP = P // 2  # 98: max-halving split

    # Bacc (not plain Bass): its compile() runs move_matmul_waits_to_ldweights
    # + generate_event_semaphores, which legalize multi-semaphore waits for
    # the 1-wait-per-instruction ISA constraint.
    nc = bacc.Bacc("TRN2", target_bir_lowering=False, debug=False)

    mov_d = nc.dram_tensor("mov", [D, 4 * P], u8, kind="ExternalInput")
    stat_nn_d = nc.dram_tensor("stat_nn", [D, NN_STAT_COLS], u8, kind="ExternalInput")
    stat_nd_d = nc.dram_tensor("stat_nd", [D, ND_STAT_COLS], u8, kind="ExternalInput")
    wmask_d = nc.dram_tensor("wmask", [128, NN_COLS], f32, kind="ExternalInput")
    out_d = nc.dram_tensor("out", [1, 2], f32, kind="ExternalOutput")

    red_mode = _red_mode_seq(len(SCHED))

    with tile.TileContext(nc) as tc:
        with (
            tc.tile_pool(name="const", bufs=1) as const_pool,
            tc.tile_pool(name="statp", bufs=1) as stat_pool,
            tc.tile_pool(name="slots", bufs=1) as slot_pool,
            tc.tile_pool(name="stageB", bufs=3) as stageB_pool,
            tc.tile_pool(name="stageC", bufs=3) as stageC_pool,
            tc.tile_pool(name="psum", bufs=3, space="PSUM") as psum_pool,
            tc.tile_pool(name="psum_f", bufs=1, space="PSUM") as psum_f_pool,
        ):
            # Moving operand first: every unit needs it.
            mov_sb = const_pool.tile([128, 6, 4 * P], u8)
            nc.sync.dma_start(
                mov_sb[:], mov_d[:, :].rearrange("(c k) p -> k c p", k=128)
            )

            wmask_sb = const_pool.tile([128, NN_COLS], f32)
            nc.sync.dma_start(wmask_sb[:], wmask_d[:, :])
            ones_sb = const_pool.tile([128, 1], f32)
            nc.vector.memset(ones_sb[:], 1.0)

            stat_nn_sb = stat_pool.tile([128, 6, NN_STAT_COLS], u8)
            stat_nd_sb = stat_pool.tile([128, 6, ND_STAT_COLS], u8)
            # Chunked stationary DMAs, in consumption order (nn then nd),
            # first chunk small so compute starts early.
            def _stat_chunks(dram, sbuf, ncols, bounds):
                for lo, hi in zip(bounds[:-1], bounds[1:]):
                    src = dram[:, 128 * lo : 128 * hi].rearrange(
                        "(c k) q -> k c q", k=128
                    )
                    nc.sync.dma_start(sbuf[:, :, 128 * lo : 128 * hi], src)

            nn_slots = NN_STAT_COLS // 128
            _stat_chunks(stat_nn_d, stat_nn_sb, NN_STAT_COLS,
                         [0, 4, 12, 24, 36, nn_slots])
            _stat_chunks(stat_nd_d, stat_nd_sb, ND_STAT_COLS,
                         [0, 12, 25, 38, NT])
            stat_sbs = (stat_nn_sb, stat_nd_sb)

            mslots = slot_pool.tile([128, NCOLS], f32, name="mslots")
            nc.vector.memset(mslots[:], 0.0)
            acc2 = const_pool.tile([128, 2], f32)

            for u, unit in enumerate(SCHED):
                ps = psum_pool.tile([128, 2, 512], f32, tag="ps")
                nb = len(unit["banks"])
                for b, (slot, half) in enumerate(unit["banks"]):
                    stat_sb = stat_sbs[unit["side"]]
                    for t3 in range(3):
                        lhsT = stat_sb[
                            :, 2 * t3 : 2 * t3 + 2, 128 * slot : 128 * (slot + 1)
                        ].bitcast(f8)
                        rhs = mov_sb[
                            :, 2 * t3 : 2 * t3 + 2, 392 * half : 392 * half + 392
                        ].bitcast(f8)
                        nc.tensor.matmul(
                            ps[:, b, 0:392],
                            lhsT,
                            rhs,
                            start=(t3 == 0),
                            stop=(t3 == 2),
                            perf_mode=DR,
                        )
                mview = ps[:, 0:nb, 0:392].rearrange("k b (i p) -> k b i p", p=P)
                mout = mslots[:, 4 * u : 4 * u + 2 * nb]
                mode = red_mode[u]
                if mode == "V":
                    nc.vector.reduce_max(
                        out=mout, in_=mview, axis=mybir.AxisListType.X
                    )
                elif mode == "B":
                    h = stageB_pool.tile([128, 2, 2, HP], f32, tag="hB")
                    nc.gpsimd.tensor_max(
                        h[:, 0:nb], mview[:, :, :, 0:HP], mview[:, :, :, HP:P]
                    )
                    nc.vector.reduce_max(
                        out=mout, in_=h[:, 0:nb], axis=mybir.AxisListType.X
                    )
                else:  # "C"
                    hc = stageC_pool.tile([128, 2, 2, P], bf16, tag="hC")
                    nc.scalar.copy(hc[:, 0:nb], mview)
                    h2 = stageC_pool.tile([128, 2, 2, HP], bf16, tag="hC2")
                    nc.vector.tensor_max(
                        h2[:, 0:nb], hc[:, 0:nb, :, 0:HP], hc[:, 0:nb, :, HP:P]
                    )
                    nc.vector.reduce_max(
                        out=mout, in_=h2[:, 0:nb], axis=mybir.AxisListType.X
                    )

            # pos: acc2[:,0] = sum_q mask * m  (nn columns, fused accum)
            junk = slot_pool.tile([128, NN_COLS], f32, name="junk0")
            nc.vector.scalar_tensor_tensor(
                out=junk[:],
                in0=mslots[:, 0:NN_COLS],
                scalar=1.0,
                in1=wmask_sb[:],
                op0=mybir.AluOpType.mult,
                op1=mybir.AluOpType.mult,
                accum_out=acc2[:, 0:1],
            )
            # neg: acc2[:,1] = sum_q relu(m - margin*S^2)  (nd columns)
            junk2 = slot_pool.tile([128, NCOLS - NN_COLS], f32, name="junk1")
            nc.vector.tensor_scalar(
                out=junk2[:],
                in0=mslots[:, NN_COLS:NCOLS],
                scalar1=-MARGIN * SCALE * SCALE,
                scalar2=0.0,
                op0=mybir.AluOpType.add,
                op1=mybir.AluOpType.max,
                accum_out=acc2[:, 1:2],
            )
            # partition reduction: [1,2] = ones[128,1].T @ acc2[128,2]
            ps_f = psum_f_pool.tile([1, 2], f32)
            nc.tensor.matmul(ps_f[:], ones_sb[:], acc2[:], start=True, stop=True)
            out_sb = const_pool.tile([1, 2], f32)
            nc.vector.tensor_copy(out_sb[:], ps_f[:])
            nc.sync.dma_start(out_d[:, :], out_sb[:])

    nc.compile()
    return nc


def _quant(n):
    """[*, D] fp32 normalized -> fp8e4m3 bytes of n*SCALE."""
    import ml_dtypes

    return (n * SCALE).astype(ml_dtypes.float8_e4m3).view(np.uint8)


def _statq(n):
    """[32,P,D] -> d-major [D, 32*P] (j-major q axis)."""
    return np.ascontiguousarray(n.transpose(2, 0, 1).reshape(D, N_IMG * P))


def _build_in_maps(normal_embed, defect_embed):
    x1 = np.asarray(normal_embed, dtype=np.float32)
    x2 = np.asarray(defect_embed, dtype=np.float32)
    n1 = x1 / (np.sqrt(np.sum(x1 * x1, axis=-1, keepdims=True)) + EPS)
    n2 = x2 / (np.sqrt(np.sum(x2 * x2, axis=-1, keepdims=True)) + EPS)

    q1 = _statq(_quant(n1))  # [D, 6272] uint8 view of fp8
    q2 = _statq(_quant(n2))

    jq = np.arange(NT * 128) // P  # j image per stationary q

    in_maps = []
    for c in range(NCORES):
        pa, pb = _pairs(c)
        imgs = [pa[0], pa[1], pb[0], pb[1]]
        mov = np.ascontiguousarray(
            np.concatenate([q1[:, i * P : (i + 1) * P] for i in imgs], axis=1)
        )

        if TRI:
            tiles = list(range(FIXED_A_T0, NT)) + _flex_tiles(c)
            stat_nn = np.ascontiguousarray(
                np.concatenate(
                    [q1[:, 128 * t : 128 * (t + 1)] for t in tiles], axis=1
                )
            )
            slot2tile = tiles
        else:
            stat_nn = q1
            slot2tile = list(range(NT))

        wm = np.zeros((128, NN_COLS), dtype=np.float32)
        for u, unit in enumerate(SCHED):
            if unit["side"] != 0:
                continue
            for b, (slot, half) in enumerate(unit["banks"]):
                t = slot2tile[slot]
                pair = pa if half == 0 else pb
                # flex double-count guard: pair A contributions only from
                # tiles below the fixed-A section, pair B only from tiles
                # below fixed-B but not in fixed-A's pair-A... (fixed
                # sections themselves are the unique cover for their range)
                if TRI and u >= NN_UNITS - N_FLEX:
                    ok = (t < FIXED_A_T0) if half == 0 else (t >= FIXED_A_T0)
                    if not ok:
                        continue
                q = 128 * t + np.arange(128)
                for m in range(2):
                    col = 4 * u + 2 * b + m
                    wm[:, col] = (jq[q] > pair[m]).astype(np.float32)
        assert int(wm.sum()) == 62 * P, (c, int(wm.sum()))

        in_maps.append(
            {
                "mov": mov,
                "stat_nn": stat_nn,
                "stat_nd": q2,
                "wmask": np.ascontiguousarray(wm),
            }
        )
    return in_maps


def _get_nc():
    key = ("nc", TRI, RED_PATTERN)
    if key not in _CACHE:
        _CACHE[key] = _build_nc()
    return _CACHE[key]


def _run_on_device(in_maps, trace=False):
    from concourse.bass_utils import run_bass_kernel_spmd

    nc = _get_nc()
    return run_bass_kernel_spmd(
        nc, in_maps, core_ids=list(range(NCORES)), trace=trace
    )


def _combine(results):
    s_pos = 0.0
    s_neg = 0.0
    for r in results:
        o = np.asarray(r["out"], dtype=np.float64)
        s_pos += float(o[0, 0])
        s_neg += float(o[0, 1])
    s2 = SCALE * SCALE
    loss = 1.0 - s_pos / s2 / (NPAIRS * P) + s_neg / s2 / (N_IMG * N_IMG * P)
    return np.float32(loss)


def kernel(normal_embed, defect_embed):
    in_maps = _build_in_maps(normal_embed, defect_embed)
    res = _run_on_device(in_maps, trace=False)
    return _combine(res.results)


# revision 10
# speedup vs baseline: 3.8313x; 1.0908x over previous
"""Trainium2 Bass kernel for nn_ContrastiveLoss (patch-level contrastive loss).

Reference math:
  n1 = normalize(normal_embed)  [N,P,D], n2 = normalize(defect_embed) [M,P,D]
  sim_nn[i,j,q] = max_p <n1[i,p,:], n1[j,q,:]>   (max over first arg's patches)
  sim_nd[i,j,q] = max_p <n1[i,p,:], n2[j,q,:]>
  pos_loss = sum_{i<j,q} (1 - sim_nn[i,j,q]) / (npairs*P)
  neg_loss = mean(relu(sim_nd - 0.5))
  loss = pos_loss + neg_loss

Distribution (8 NeuronCores, data-parallel over i):
  Core c owns moving pairs A=(2c, 2c+1), B=(30-2c, 31-2c). Embeddings are
  normalized on host, scaled by S, quantized to fp8e4m3 and shipped as
  uint8 (bitcast to float8e4 at the matmul). Matmuls run in DoubleRow perf
  mode: each instruction contracts TWO 128-deep k-chunks ([128,2,*] APs),
  so D=768 takes 3 matmuls per 392-wide PSUM bank.

  The j-side streams as 128-wide stationary q-tiles against 392-wide moving
  halves (one image pair per bank, two banks per unit). The nn side
  exploits the i<j triangle with a core-uniform schedule (single SPMD
  program):
    - flex: 24 host-packed per-core q-tiles vs BOTH pairs (exactly the
      per-core leftovers: (25-tA)+(47-tB) == 24 for every core); the wmask
      kills the half that doesn't apply. Runs FIRST: flex consumes one
      stationary slot per unit, letting the DMA stream get ahead.
    - fixed-A: q-tiles 25..48 vs pair A (every core's pair A needs them all
      since max_c tileof(2c+1) = 22 < 25),
    - fixed-B: q-tiles 47,48 vs pair B,
  plus the full 49-tile sweep for the nd side.

  Max-over-p runs straight out of PSUM on a rotating mix of engines
  (pattern-tunable): B = GpSimd tensor_max halves PSUM->SBUF f32 and DVE
  reduce_max finishes; C = Act copies PSUM->SBUF bf16 and DVE tensor_max +
  reduce_max finish; D = Act copies, GpSimd halves, DVE finishes; V = DVE
  reduce_max direct. Warmup matmuls on zeros ramp the PE p-state during the
  initial DMA fill. Finals are segmented (fused multiply/relu-accumulate)
  so only the last segment sits in the tail; a ones-matmul folds partitions
  and the host combines per-core partials.
"""

import os

import numpy as np

# Problem constants (hardcoded per the contract; kernel.py is self-contained).
N_IMG = 32
P = 196
D = 768
EPS = 1e-8
MARGIN = 0.5
NCORES = 8
NT = N_IMG * P // 128       # 49 stationary q-tiles per side (exact: 6272/128)
NPAIRS = N_IMG * (N_IMG - 1) // 2

# fp8 scale: sims come out multiplied by SCALE^2; undone on host.
SCALE = 16.0

# Triangular-nn schedule (1) vs full NxN (0).
TRI = os.environ.get("CL_TRI", "1") == "1"
# Reduce pipeline mode per unit, cyclic pattern (see module docstring).
RED_PATTERN = os.environ.get("CL_RED", "BCBBDBCBBCBBD")
# PE p-state warmup matmuls issued before the first real unit.
WARM = int(os.environ.get("CL_WARM", "16"))
# PSUM pool depth (tiles of 2 banks each; 4 uses all 8 banks).
PSUM_BUFS = int(os.environ.get("CL_PSUM", "4"))

# Per-core pair bases: pair A = (2c, 2c+1), pair B = (30-2c, 31-2c).
TA = [((2 * c + 1) * P) // 128 for c in range(NCORES)]   # first tile pair A needs
TB = [((31 - 2 * c) * P) // 128 for c in range(NCORES)]  # first tile pair B needs
FIXED_A_T0 = 25   # fixed-A section: tiles 25..48 (>= max(TA)=22)
FIXED_B_T0 = 47   # fixed-B section: tiles 47,48 (>= max(TB)=47)
N_FIXED_A = NT - FIXED_A_T0          # 24
N_FLEX = 24
for _c in range(NCORES):
    assert (FIXED_A_T0 - TA[_c]) + (FIXED_B_T0 - TB[_c]) == N_FLEX
NN_SLOTS = N_FLEX + N_FIXED_A        # 48 packed stationary nn tiles

_CACHE = {}


def _pairs(c):
    return (2 * c, 2 * c + 1), (30 - 2 * c, 31 - 2 * c)


def _flex_tiles(c):
    """Per-core flex q-tiles: pair-A leftovers then pair-B leftovers."""
    return list(range(TA[c], FIXED_A_T0)) + list(range(TB[c], FIXED_B_T0))


def _schedule():
    """Core-independent unit list. Each unit: side 0 nn / 1 nd; banks: list
    of (slot, half); half 0 = moving cols 0:392 (pair A), 1 = 392:784 (pair
    B). Slot indexes 128-col groups of that side's packed stationary."""
    units = []
    if TRI:
        # flex first: slots 0..23, both pairs (1 slot per unit)
        for k in range(N_FLEX):
            units.append({"side": 0, "banks": [(k, 0), (k, 1)]})
        # fixed-A: slots 24..47 = tiles 25..48, pair A (2 slots per unit)
        for u in range(N_FIXED_A // 2):
            s = N_FLEX + 2 * u
            units.append({"side": 0, "banks": [(s, 0), (s + 1, 0)]})
        # fixed-B: tiles 47,48 = slots 46,47, pair B
        units.append({"side": 0, "banks": [(NN_SLOTS - 2, 1), (NN_SLOTS - 1, 1)]})
    else:
        for t in range(NT):
            units.append({"side": 0, "banks": [(t, 0), (t, 1)]})
    for t in range(NT):
        units.append({"side": 1, "banks": [(t, 0), (t, 1)]})
    return units


SCHED = _schedule()
NN_UNITS = sum(1 for u in SCHED if u["side"] == 0)
NN_COLS = 4 * NN_UNITS
NCOLS = 4 * len(SCHED)
NN_STAT_COLS = (NN_SLOTS if TRI else NT) * 128
ND_STAT_COLS = NT * 128
# final-sum segments: (first_unit, last_unit_exclusive, is_pos)
ND0 = NN_UNITS
_NDMID = ND0 + (len(SCHED) - ND0) // 2
SEGMENTS = [(0, NN_UNITS, True), (ND0, _NDMID, False), (_NDMID, len(SCHED), False)]


def _red_mode_seq(n):
    return [RED_PATTERN[u % len(RED_PATTERN)] for u in range(n)]


def _build_nc():
    import concourse.bacc as bacc
    import concourse.mybir as mybir
    import concourse.tile as tile

    f32 = mybir.dt.float32
    bf16 = mybir.dt.bfloat16
    u8 = mybir.dt.uint8
    f8 = mybir.dt.float8e4
    DR = mybir.MatmulPerfMode.DoubleRow
    HP = P // 2  # 98: max-halving split

    # Bacc (not plain Bass): its compile() runs move_matmul_waits_to_ldweights
    # + generate_event_semaphores, which legalize multi-semaphore waits for
    # the 1-wait-per-instruction ISA constraint.
    nc = bacc.Bacc("TRN2", target_bir_lowering=False, debug=False)

    mov_d = nc.dram_tensor("mov", [D, 4 * P], u8, kind="ExternalInput")
    stat_nn_d = nc.dram_tensor("stat_nn", [D, NN_STAT_COLS], u8, kind="ExternalInput")
    stat_nd_d = nc.dram_tensor("stat_nd", [D, ND_STAT_COLS], u8, kind="ExternalInput")
    wmask_d = nc.dram_tensor("wmask", [128, NN_COLS], f32, kind="ExternalInput")
    out_d = nc.dram_tensor("out", [1, len(SEGMENTS)], f32, kind="ExternalOutput")

    red_mode = _red_mode_seq(len(SCHED))

    with tile.TileContext(nc) as tc:
        with (
            tc.tile_pool(name="const", bufs=1) as const_pool,
            tc.tile_pool(name="statp", bufs=1) as stat_pool,
            tc.tile_pool(name="slots", bufs=1) as slot_pool,
            tc.tile_pool(name="stageB", bufs=3) as stageB_pool,
            tc.tile_pool(name="stageC", bufs=3) as stageC_pool,
            tc.tile_pool(name="psum", bufs=PSUM_BUFS, space="PSUM") as psum_pool,
        ):
            # Moving operand first: every unit needs it.
            mov_sb = const_pool.tile([128, 6, 4 * P], u8)
            nc.sync.dma_start(
                mov_sb[:], mov_d[:, :].rearrange("(c k) p -> k c p", k=128)
            )

            stat_nn_sb = stat_pool.tile([128, 6, NN_STAT_COLS], u8)
            stat_nd_sb = stat_pool.tile([128, 6, ND_STAT_COLS], u8)

            # Chunked stationary DMAs in consumption order; first chunk small
            # so compute starts early. wmask last (needed only at finals).
            def _stat_chunks(dram, sbuf, bounds):
                for lo, hi in zip(bounds[:-1], bounds[1:]):
                    src = dram[:, 128 * lo : 128 * hi].rearrange(
                        "(c k) q -> k c q", k=128
                    )
                    nc.sync.dma_start(sbuf[:, :, 128 * lo : 128 * hi], src)

            _stat_chunks(stat_nn_d, stat_nn_sb,
                         [0, 4, 12, 24, 36, NN_STAT_COLS // 128])
            _stat_chunks(stat_nd_d, stat_nd_sb, [0, 12, 25, 37, NT])
            stat_sbs = (stat_nn_sb, stat_nd_sb)

            wmask_sb = const_pool.tile([128, NN_COLS], f32)
            nc.sync.dma_start(wmask_sb[:], wmask_d[:, :])

            ones_sb = const_pool.tile([128, 1], f32)
            nc.vector.memset(ones_sb[:], 1.0)
            mslots = slot_pool.tile([128, NCOLS], f32, name="mslots")
            acc = const_pool.tile([128, len(SEGMENTS)], f32)

            # PE p-state warmup on zeros while the first DMAs land.
            if WARM:
                zeros_sb = const_pool.tile([128, 2, 392], u8)
                nc.vector.memset(zeros_sb[:], 0)
                pw = psum_pool.tile([128, 2, 512], f32, tag="ps")
                for _ in range(WARM):
                    nc.tensor.matmul(
                        pw[:, 0, 0:392],
                        zeros_sb[:, :, 0:128].bitcast(f8),
                        zeros_sb[:].bitcast(f8),
                        start=True,
                        stop=True,
                        perf_mode=DR,
                    )

            seg_of_unit = {}
            for si, (u0, u1, _) in enumerate(SEGMENTS):
                for u in range(u0, u1):
                    seg_of_unit[u] = si
            junk = slot_pool.tile([128, NN_COLS], f32, name="junk")

            for u, unit in enumerate(SCHED):
                ps = psum_pool.tile([128, 2, 512], f32, tag="ps")
                for b, (slot, half) in enumerate(unit["banks"]):
                    stat_sb = stat_sbs[unit["side"]]
                    for t3 in range(3):
                        lhsT = stat_sb[
                            :, 2 * t3 : 2 * t3 + 2, 128 * slot : 128 * (slot + 1)
                        ].bitcast(f8)
                        rhs = mov_sb[
                            :, 2 * t3 : 2 * t3 + 2, 392 * half : 392 * half + 392
                        ].bitcast(f8)
                        nc.tensor.matmul(
                            ps[:, b, 0:392],
                            lhsT,
                            rhs,
                            start=(t3 == 0),
                            stop=(t3 == 2),
                            perf_mode=DR,
                        )
                mview = ps[:, :, 0:392].rearrange("k b (i p) -> k b i p", p=P)
                mout = mslots[:, 4 * u : 4 * u + 4]
                mode = red_mode[u]
                if mode == "V":
                    nc.vector.reduce_max(
                        out=mout, in_=mview, axis=mybir.AxisListType.X
                    )
                elif mode == "B":
                    h = stageB_pool.tile([128, 2, 2, HP], f32, tag="hB")
                    nc.gpsimd.tensor_max(
                        h[:], mview[:, :, :, 0:HP], mview[:, :, :, HP:P]
                    )
                    nc.vector.reduce_max(
                        out=mout, in_=h[:], axis=mybir.AxisListType.X
                    )
                else:  # "C" / "D"
                    hc = stageC_pool.tile([128, 2, 2, P], bf16, tag="hC")
                    nc.scalar.copy(hc[:], mview)
                    h2 = stageC_pool.tile([128, 2, 2, HP], bf16, tag="hC2")
                    eng = nc.vector if mode == "C" else nc.gpsimd
                    eng.tensor_max(
                        h2[:], hc[:, :, :, 0:HP], hc[:, :, :, HP:P]
                    )
                    nc.vector.reduce_max(
                        out=mout, in_=h2[:], axis=mybir.AxisListType.X
                    )

                # segment finals, fused accumulate, off the tail
                for si, (u0, u1, is_pos) in enumerate(SEGMENTS):
                    if u != u1 - 1:
                        continue
                    c0, c1 = 4 * u0, 4 * u1
                    if is_pos:
                        nc.vector.scalar_tensor_tensor(
                            out=junk[:, 0 : c1 - c0],
                            in0=mslots[:, c0:c1],
                            scalar=1.0,
                            in1=wmask_sb[:, c0:c1],
                            op0=mybir.AluOpType.mult,
                            op1=mybir.AluOpType.mult,
                            accum_out=acc[:, si : si + 1],
                        )
                    else:
                        nc.vector.tensor_scalar(
                            out=junk[:, 0 : c1 - c0],
                            in0=mslots[:, c0:c1],
                            scalar1=-MARGIN * SCALE * SCALE,
                            scalar2=0.0,
                            op0=mybir.AluOpType.add,
                            op1=mybir.AluOpType.max,
                            accum_out=acc[:, si : si + 1],
                        )

            # partition reduction via ones-matmul into a rotated psum tile
            ps_f = psum_pool.tile([128, 2, 512], f32, tag="ps")
            nc.tensor.matmul(
                ps_f[0:1, 0, 0 : len(SEGMENTS)],
                ones_sb[:],
                acc[:],
                start=True,
                stop=True,
            )
            out_sb = const_pool.tile([1, len(SEGMENTS)], f32)
            nc.vector.tensor_copy(out_sb[:], ps_f[0:1, 0, 0 : len(SEGMENTS)])
            nc.sync.dma_start(out_d[:, :], out_sb[:])

    nc.compile()
    return nc


def _quant(n):
    """[*, D] fp32 normalized -> fp8e4m3 bytes of n*SCALE."""
    import ml_dtypes

    return (n * SCALE).astype(ml_dtypes.float8_e4m3).view(np.uint8)


def _statq(n):
    """[32,P,D] -> d-major [D, 32*P] (j-major q axis)."""
    return np.ascontiguousarray(n.transpose(2, 0, 1).reshape(D, N_IMG * P))


def _build_in_maps(normal_embed, defect_embed):
    x1 = np.asarray(normal_embed, dtype=np.float32)
    x2 = np.asarray(defect_embed, dtype=np.float32)
    n1 = x1 / (np.sqrt(np.sum(x1 * x1, axis=-1, keepdims=True)) + EPS)
    n2 = x2 / (np.sqrt(np.sum(x2 * x2, axis=-1, keepdims=True)) + EPS)

    q1 = _statq(_quant(n1))  # [D, 6272] uint8 view of fp8
    q2 = _statq(_quant(n2))

    jq = np.arange(NT * 128) // P  # j image per stationary q

    in_maps = []
    for c in range(NCORES):
        pa, pb = _pairs(c)
        imgs = [pa[0], pa[1], pb[0], pb[1]]
        mov = np.ascontiguousarray(
            np.concatenate([q1[:, i * P : (i + 1) * P] for i in imgs], axis=1)
        )

        if TRI:
            tiles = _flex_tiles(c) + list(range(FIXED_A_T0, NT))
            stat_nn = np.ascontiguousarray(
                np.concatenate(
                    [q1[:, 128 * t : 128 * (t + 1)] for t in tiles], axis=1
                )
            )
            slot2tile = tiles
        else:
            stat_nn = q1
            slot2tile = list(range(NT))

        wm = np.zeros((128, NN_COLS), dtype=np.float32)
        for u, unit in enumerate(SCHED):
            if unit["side"] != 0:
                continue
            for b, (slot, half) in enumerate(unit["banks"]):
                t = slot2tile[slot]
                pair = pa if half == 0 else pb
                # flex double-count guard: the fixed sections are the unique
                # cover for tiles >= FIXED_A_T0 (pair A) / >= FIXED_B_T0
                # (pair B), so flex contributes pair A only below FIXED_A_T0
                # and pair B only at/above it (flex-B tiles are < FIXED_B_T0).
                if TRI and u < N_FLEX:
                    ok = (t < FIXED_A_T0) if half == 0 else (t >= FIXED_A_T0)
                    if not ok:
                        continue
                q = 128 * t + np.arange(128)
                for m in range(2):
                    col = 4 * u + 2 * b + m
                    wm[:, col] = (jq[q] > pair[m]).astype(np.float32)
        assert int(wm.sum()) == 62 * P, (c, int(wm.sum()))

        in_maps.append(
            {
                "mov": mov,
                "stat_nn": stat_nn,
                "stat_nd": q2,
                "wmask": np.ascontiguousarray(wm),
            }
        )
    return in_maps


def _get_nc():
    key = ("nc", TRI, RED_PATTERN, WARM, PSUM_BUFS)
    if key not in _CACHE:
        _CACHE[key] = _build_nc()
    return _CACHE[key]


def _run_on_device(in_maps, trace=False):
    from concourse.bass_utils import run_bass_kernel_spmd

    nc = _get_nc()
    return run_bass_kernel_spmd(
        nc, in_maps, core_ids=list(range(NCORES)), trace=trace
    )


def _combine(results):
    s_pos = 0.0
    s_neg = 0.0
    for r in results:
        o = np.asarray(r["out"], dtype=np.float64).reshape(-1)
        for si, (_, _, is_pos) in enumerate(SEGMENTS):
            if is_pos:
                s_pos += float(o[si])
            else:
                s_neg += float(o[si])
    s2 = SCALE * SCALE
    loss = 1.0 - s_pos / s2 / (NPAIRS * P) + s_neg / s2 / (N_IMG * N_IMG * P)
    return np.float32(loss)


def kernel(normal_embed, defect_embed):
    in_maps = _build_in_maps(normal_embed, defect_embed)
    res = _run_on_device(in_maps, trace=False)
    return _combine(res.results)


# revision 13
# speedup vs baseline: 3.9318x; 1.0262x over previous
"""Trainium2 Bass kernel for nn_ContrastiveLoss (patch-level contrastive loss).

Reference math:
  n1 = normalize(normal_embed)  [N,P,D], n2 = normalize(defect_embed) [M,P,D]
  sim_nn[i,j,q] = max_p <n1[i,p,:], n1[j,q,:]>   (max over first arg's patches)
  sim_nd[i,j,q] = max_p <n1[i,p,:], n2[j,q,:]>
  pos_loss = sum_{i<j,q} (1 - sim_nn[i,j,q]) / (npairs*P)
  neg_loss = mean(relu(sim_nd - 0.5))
  loss = pos_loss + neg_loss

Distribution (8 NeuronCores, data-parallel over i):
  Core c owns moving pairs A=(2c, 2c+1), B=(30-2c, 31-2c). Embeddings are
  normalized on host, scaled by S, quantized to fp8e4m3 and shipped as
  uint8 (bitcast to float8e4 at the matmul). Matmuls run in DoubleRow perf
  mode: each instruction contracts TWO 128-deep k-chunks ([128,2,*] APs),
  so D=768 takes 3 matmuls per 392-wide PSUM bank.

  The j-side streams as 128-wide stationary q-tiles against 392-wide moving
  halves (one image pair per bank, two banks per unit). The nn side
  exploits the i<j triangle with a core-uniform schedule (single SPMD
  program):
    - flex: 24 host-packed per-core q-tiles vs BOTH pairs (exactly the
      per-core leftovers: (25-tA)+(47-tB) == 24 for every core); the wmask
      kills the half that doesn't apply. Runs FIRST: flex consumes one
      stationary slot per unit, letting the DMA stream get ahead.
    - fixed-A: q-tiles 25..48 vs pair A (every core's pair A needs them all
      since max_c tileof(2c+1) = 22 < 25),
    - fixed-B: q-tiles 47,48 vs pair B,
  plus the full 49-tile sweep for the nd side.

  Max-over-p runs straight out of PSUM on a rotating mix of engines
  (pattern-tunable): B = GpSimd tensor_max halves PSUM->SBUF f32 and DVE
  reduce_max finishes; C = Act copies PSUM->SBUF bf16 and DVE tensor_max +
  reduce_max finish; D = Act copies, GpSimd halves, DVE finishes; V = DVE
  reduce_max direct. Warmup matmuls on zeros ramp the PE p-state during the
  initial DMA fill. Finals are segmented (fused multiply/relu-accumulate)
  so only the last segment sits in the tail; a ones-matmul folds partitions
  and the host combines per-core partials.
"""

import os

import numpy as np

# Problem constants (hardcoded per the contract; kernel.py is self-contained).
N_IMG = 32
P = 196
D = 768
EPS = 1e-8
MARGIN = 0.5
NCORES = 8
NT = N_IMG * P // 128       # 49 stationary q-tiles per side (exact: 6272/128)
NPAIRS = N_IMG * (N_IMG - 1) // 2

# fp8 scale: sims come out multiplied by SCALE^2; undone on host.
SCALE = 16.0

# Triangular-nn schedule (1) vs full NxN (0).
TRI = os.environ.get("CL_TRI", "1") == "1"
# Reduce pipeline mode per unit, cyclic pattern (see module docstring).
RED_PATTERN = os.environ.get("CL_RED", "BCBBDBCBBCBBD")
# PE p-state warmup matmuls issued before the first real unit.
WARM = int(os.environ.get("CL_WARM", "16"))
# PSUM pool depth (tiles of 2 banks each; 4 uses all 8 banks).
PSUM_BUFS = int(os.environ.get("CL_PSUM", "4"))

# Per-core pair bases: pair A = (2c, 2c+1), pair B = (30-2c, 31-2c).
TA = [((2 * c + 1) * P) // 128 for c in range(NCORES)]   # first tile pair A needs
TB = [((31 - 2 * c) * P) // 128 for c in range(NCORES)]  # first tile pair B needs
FIXED_A_T0 = 25   # fixed-A section: tiles 25..48 (>= max(TA)=22)
FIXED_B_T0 = 47   # fixed-B section: tiles 47,48 (>= max(TB)=47)
N_FIXED_A = NT - FIXED_A_T0          # 24
N_FLEX = 24
for _c in range(NCORES):
    assert (FIXED_A_T0 - TA[_c]) + (FIXED_B_T0 - TB[_c]) == N_FLEX
NN_SLOTS = N_FLEX + N_FIXED_A        # 48 packed stationary nn tiles

_CACHE = {}


def _pairs(c):
    return (2 * c, 2 * c + 1), (30 - 2 * c, 31 - 2 * c)


def _flex_tiles(c):
    """Per-core flex q-tiles: pair-A leftovers then pair-B leftovers."""
    return list(range(TA[c], FIXED_A_T0)) + list(range(TB[c], FIXED_B_T0))


def _schedule():
    """Core-independent unit list. Each unit: side 0 nn / 1 nd; banks: list
    of (slot, half); half 0 = moving cols 0:392 (pair A), 1 = 392:784 (pair
    B). Slot indexes 128-col groups of that side's packed stationary."""
    units = []
    if TRI:
        # flex first: slots 0..23, both pairs (1 slot per unit)
        for k in range(N_FLEX):
            units.append({"side": 0, "banks": [(k, 0), (k, 1)]})
        # fixed-A: slots 24..47 = tiles 25..48, pair A (2 slots per unit)
        for u in range(N_FIXED_A // 2):
            s = N_FLEX + 2 * u
            units.append({"side": 0, "banks": [(s, 0), (s + 1, 0)]})
        # fixed-B: tiles 47,48 = slots 46,47, pair B
        units.append({"side": 0, "banks": [(NN_SLOTS - 2, 1), (NN_SLOTS - 1, 1)]})
    else:
        for t in range(NT):
            units.append({"side": 0, "banks": [(t, 0), (t, 1)]})
    for t in range(NT):
        units.append({"side": 1, "banks": [(t, 0), (t, 1)]})
    return units


SCHED = _schedule()
NN_UNITS = sum(1 for u in SCHED if u["side"] == 0)
NN_COLS = 4 * NN_UNITS
NCOLS = 4 * len(SCHED)
NN_STAT_COLS = (NN_SLOTS if TRI else NT) * 128
ND_STAT_COLS = NT * 128
# final-sum segments: (first_unit, last_unit_exclusive, is_pos). The last
# segment is small so only its drain sits in the serial tail.
ND0 = NN_UNITS
_NDTAIL = max(ND0, len(SCHED) - 6)
SEGMENTS = [
    (0, NN_UNITS, True),
    (ND0, _NDTAIL, False),
    (_NDTAIL, len(SCHED), False),
]


def _red_mode_seq(n):
    return [RED_PATTERN[u % len(RED_PATTERN)] for u in range(n)]


def _build_nc():
    import concourse.bacc as bacc
    import concourse.mybir as mybir
    import concourse.tile as tile

    f32 = mybir.dt.float32
    bf16 = mybir.dt.bfloat16
    u8 = mybir.dt.uint8
    f8 = mybir.dt.float8e4
    DR = mybir.MatmulPerfMode.DoubleRow
    HP = P // 2  # 98: max-halving split

    # Bacc (not plain Bass): its compile() runs move_matmul_waits_to_ldweights
    # + generate_event_semaphores, which legalize multi-semaphore waits for
    # the 1-wait-per-instruction ISA constraint.
    nc = bacc.Bacc("TRN2", target_bir_lowering=False, debug=False)

    mov_d = nc.dram_tensor("mov", [D, 4 * P], u8, kind="ExternalInput")
    stat_nn_d = nc.dram_tensor("stat_nn", [D, NN_STAT_COLS], u8, kind="ExternalInput")
    stat_nd_d = nc.dram_tensor("stat_nd", [D, ND_STAT_COLS], u8, kind="ExternalInput")
    wmask_d = nc.dram_tensor("wmask", [128, NN_COLS], f32, kind="ExternalInput")
    out_d = nc.dram_tensor("out", [1, len(SEGMENTS)], f32, kind="ExternalOutput")

    red_mode = _red_mode_seq(len(SCHED))

    with tile.TileContext(nc) as tc:
        with (
            tc.tile_pool(name="const", bufs=1) as const_pool,
            tc.tile_pool(name="statp", bufs=1) as stat_pool,
            tc.tile_pool(name="slots", bufs=1) as slot_pool,
            tc.tile_pool(name="stageB", bufs=3) as stageB_pool,
            tc.tile_pool(name="stageC", bufs=3) as stageC_pool,
            tc.tile_pool(name="psum", bufs=PSUM_BUFS, space="PSUM") as psum_pool,
        ):
            # Moving operand first: every unit needs it.
            mov_sb = const_pool.tile([128, 6, 4 * P], u8)
            nc.sync.dma_start(
                mov_sb[:], mov_d[:, :].rearrange("(c k) p -> k c p", k=128)
            )

            stat_nn_sb = stat_pool.tile([128, 6, NN_STAT_COLS], u8)
            stat_nd_sb = stat_pool.tile([128, 6, ND_STAT_COLS], u8)

            # Chunked stationary DMAs in consumption order; first chunk small
            # so compute starts early. wmask last (needed only at finals).
            def _stat_chunks(dram, sbuf, bounds):
                for lo, hi in zip(bounds[:-1], bounds[1:]):
                    src = dram[:, 128 * lo : 128 * hi].rearrange(
                        "(c k) q -> k c q", k=128
                    )
                    nc.sync.dma_start(sbuf[:, :, 128 * lo : 128 * hi], src)

            _stat_chunks(stat_nn_d, stat_nn_sb,
                         [0, 4, 12, 24, 36, NN_STAT_COLS // 128])
            _stat_chunks(stat_nd_d, stat_nd_sb, [0, 12, 25, 37, NT])
            stat_sbs = (stat_nn_sb, stat_nd_sb)

            wmask_sb = const_pool.tile([128, NN_COLS], f32)
            nc.sync.dma_start(wmask_sb[:], wmask_d[:, :])

            ones_sb = const_pool.tile([128, 1], f32)
            nc.vector.memset(ones_sb[:], 1.0)
            mslots = slot_pool.tile([128, NCOLS], f32, name="mslots")
            acc = const_pool.tile([128, len(SEGMENTS)], f32)

            # PE p-state warmup on zeros while the first DMAs land.
            if WARM:
                zeros_sb = const_pool.tile([128, 2, 392], u8)
                nc.gpsimd.memset(zeros_sb[:], 0)
                pw = psum_pool.tile([128, 2, 512], f32, tag="ps")
                for _ in range(WARM):
                    nc.tensor.matmul(
                        pw[:, 0, 0:392],
                        zeros_sb[:, :, 0:128].bitcast(f8),
                        zeros_sb[:].bitcast(f8),
                        start=True,
                        stop=True,
                        perf_mode=DR,
                    )

            seg_of_unit = {}
            for si, (u0, u1, _) in enumerate(SEGMENTS):
                for u in range(u0, u1):
                    seg_of_unit[u] = si
            max_seg = max(4 * (u1 - u0) for u0, u1, _ in SEGMENTS)
            junk = slot_pool.tile([128, max_seg], f32, name="junk")

            for u, unit in enumerate(SCHED):
                ps = psum_pool.tile([128, 2, 512], f32, tag="ps")
                for b, (slot, half) in enumerate(unit["banks"]):
                    stat_sb = stat_sbs[unit["side"]]
                    for t3 in range(3):
                        lhsT = stat_sb[
                            :, 2 * t3 : 2 * t3 + 2, 128 * slot : 128 * (slot + 1)
                        ].bitcast(f8)
                        rhs = mov_sb[
                            :, 2 * t3 : 2 * t3 + 2, 392 * half : 392 * half + 392
                        ].bitcast(f8)
                        nc.tensor.matmul(
                            ps[:, b, 0:392],
                            lhsT,
                            rhs,
                            start=(t3 == 0),
                            stop=(t3 == 2),
                            perf_mode=DR,
                        )
                mview = ps[:, :, 0:392].rearrange("k b (i p) -> k b i p", p=P)
                mout = mslots[:, 4 * u : 4 * u + 4]
                mode = red_mode[u]
                if mode == "V":
                    nc.vector.reduce_max(
                        out=mout, in_=mview, axis=mybir.AxisListType.X
                    )
                elif mode == "B":
                    h = stageB_pool.tile([128, 2, 2, HP], f32, tag="hB")
                    nc.gpsimd.tensor_max(
                        h[:], mview[:, :, :, 0:HP], mview[:, :, :, HP:P]
                    )
                    nc.vector.reduce_max(
                        out=mout, in_=h[:], axis=mybir.AxisListType.X
                    )
                else:  # "C" / "D"
                    hc = stageC_pool.tile([128, 2, 2, P], bf16, tag="hC")
                    nc.scalar.copy(hc[:], mview)
                    h2 = stageC_pool.tile([128, 2, 2, HP], bf16, tag="hC2")
                    eng = nc.vector if mode == "C" else nc.gpsimd
                    eng.tensor_max(
                        h2[:], hc[:, :, :, 0:HP], hc[:, :, :, HP:P]
                    )
                    nc.vector.reduce_max(
                        out=mout, in_=h2[:], axis=mybir.AxisListType.X
                    )

                # segment finals, fused accumulate, off the tail
                for si, (u0, u1, is_pos) in enumerate(SEGMENTS):
                    if u != u1 - 1:
                        continue
                    c0, c1 = 4 * u0, 4 * u1
                    if is_pos:
                        nc.vector.scalar_tensor_tensor(
                            out=junk[:, 0 : c1 - c0],
                            in0=mslots[:, c0:c1],
                            scalar=1.0,
                            in1=wmask_sb[:, c0:c1],
                            op0=mybir.AluOpType.mult,
                            op1=mybir.AluOpType.mult,
                            accum_out=acc[:, si : si + 1],
                        )
                    else:
                        nc.vector.tensor_scalar(
                            out=junk[:, 0 : c1 - c0],
                            in0=mslots[:, c0:c1],
                            scalar1=-MARGIN * SCALE * SCALE,
                            scalar2=0.0,
                            op0=mybir.AluOpType.add,
                            op1=mybir.AluOpType.max,
                            accum_out=acc[:, si : si + 1],
                        )

            # partition reduction via ones-matmul into a rotated psum tile
            ps_f = psum_pool.tile([128, 2, 512], f32, tag="ps")
            nc.tensor.matmul(
                ps_f[0:1, 0, 0 : len(SEGMENTS)],
                ones_sb[:],
                acc[:],
                start=True,
                stop=True,
            )
            out_sb = const_pool.tile([1, len(SEGMENTS)], f32)
            nc.vector.tensor_copy(out_sb[:], ps_f[0:1, 0, 0 : len(SEGMENTS)])
            nc.sync.dma_start(out_d[:, :], out_sb[:])

    nc.compile()
    return nc


def _quant(n):
    """[*, D] fp32 normalized -> fp8e4m3 bytes of n*SCALE."""
    import ml_dtypes

    return (n * SCALE).astype(ml_dtypes.float8_e4m3).view(np.uint8)


def _statq(n):
    """[32,P,D] -> d-major [D, 32*P] (j-major q axis)."""
    return np.ascontiguousarray(n.transpose(2, 0, 1).reshape(D, N_IMG * P))


def _build_in_maps(normal_embed, defect_embed):
    x1 = np.asarray(normal_embed, dtype=np.float32)
    x2 = np.asarray(defect_embed, dtype=np.float32)
    n1 = x1 / (np.sqrt(np.sum(x1 * x1, axis=-1, keepdims=True)) + EPS)
    n2 = x2 / (np.sqrt(np.sum(x2 * x2, axis=-1, keepdims=True)) + EPS)

    q1 = _statq(_quant(n1))  # [D, 6272] uint8 view of fp8
    q2 = _statq(_quant(n2))

    jq = np.arange(NT * 128) // P  # j image per stationary q

    in_maps = []
    for c in range(NCORES):
        pa, pb = _pairs(c)
        imgs = [pa[0], pa[1], pb[0], pb[1]]
        mov = np.ascontiguousarray(
            np.concatenate([q1[:, i * P : (i + 1) * P] for i in imgs], axis=1)
        )

        if TRI:
            tiles = _flex_tiles(c) + list(range(FIXED_A_T0, NT))
            stat_nn = np.ascontiguousarray(
                np.concatenate(
                    [q1[:, 128 * t : 128 * (t + 1)] for t in tiles], axis=1
                )
            )
            slot2tile = tiles
        else:
            stat_nn = q1
            slot2tile = list(range(NT))

        wm = np.zeros((128, NN_COLS), dtype=np.float32)
        for u, unit in enumerate(SCHED):
            if unit["side"] != 0:
                continue
            for b, (slot, half) in enumerate(unit["banks"]):
                t = slot2tile[slot]
                pair = pa if half == 0 else pb
                # flex double-count guard: the fixed sections are the unique
                # cover for tiles >= FIXED_A_T0 (pair A) / >= FIXED_B_T0
                # (pair B), so flex contributes pair A only below FIXED_A_T0
                # and pair B only at/above it (flex-B tiles are < FIXED_B_T0).
                if TRI and u < N_FLEX:
                    ok = (t < FIXED_A_T0) if half == 0 else (t >= FIXED_A_T0)
                    if not ok:
                        continue
                q = 128 * t + np.arange(128)
                for m in range(2):
                    col = 4 * u + 2 * b + m
                    wm[:, col] = (jq[q] > pair[m]).astype(np.float32)
        assert int(wm.sum()) == 62 * P, (c, int(wm.sum()))

        in_maps.append(
            {
                "mov": mov,
                "stat_nn": stat_nn,
                "stat_nd": q2,
                "wmask": np.ascontiguousarray(wm),
            }
        )
    return in_maps


def _get_nc():
    key = ("nc", TRI, RED_PATTERN, WARM, PSUM_BUFS)
    if key not in _CACHE:
        _CACHE[key] = _build_nc()
    return _CACHE[key]


def _run_on_device(in_maps, trace=False):
    from concourse.bass_utils import run_bass_kernel_spmd

    nc = _get_nc()
    return run_bass_kernel_spmd(
        nc, in_maps, core_ids=list(range(NCORES)), trace=trace
    )


def _combine(results):
    s_pos = 0.0
    s_neg = 0.0
    for r in results:
        o = np.asarray(r["out"], dtype=np.float64).reshape(-1)
        for si, (_, _, is_pos) in enumerate(SEGMENTS):
            if is_pos:
                s_pos += float(o[si])
            else:
                s_neg += float(o[si])
    s2 = SCALE * SCALE
    loss = 1.0 - s_pos / s2 / (NPAIRS * P) + s_neg / s2 / (N_IMG * N_IMG * P)
    return np.float32(loss)


def kernel(normal_embed, defect_embed):
    in_maps = _build_in_maps(normal_embed, defect_embed)
    res = _run_on_device(in_maps, trace=False)
    return _combine(res.results)
